# revision 4
# baseline (speedup 1.0000x reference)
"""Trainium2 Bass kernel for nn_CAWN2 (scatter_memory).

Computes, for a batch of B=131072 edges:
    time = cos(cut_time * basis_freq + phase)                 [B, 128]
    agg  = [node[src]+node[tgt] | time | edge[e]]             [B, 384]
    gates = agg @ w_ih.T + b_ih + b_hh  (i, f, g, o blocks)   [B, 1536]
    c = sigmoid(i) * tanh(g);  h = sigmoid(o) * tanh(c)
returns (h, c), each [B, 384] f32.

Strategy: data-parallel over 8 NeuronCores (16384 edges each, 128 tiles of
128).  Gathers via gpsimd indirect DMA (one row per partition per
instruction).  The f-gate is never used (c0 = 0), so only the i/g/o weight
rows are shipped (1152 gate cols).  The bias is folded into the time-encode
weight rows whose cos() output is constant 1.0 for every input in range
(freq_k * max_ct + |phase_k| tiny) - verified on the actual inputs at build
time, with a DVE broadcast-add fallback.  cos(x) = sin(x + pi/2) on the ACT
engine with per-partition scale=freq / bias=phase+pi/2, computed directly in
the transposed [time_dim, batch] layout the matmul needs.
"""

import os
import sys

sys.path.insert(0, "/opt/trn_rl_repo")

import numpy as np

from concourse import bass, bacc, mybir
import concourse.tile as tile
from concourse.bass_utils import run_bass_kernel_spmd
from concourse.masks import make_identity

NCORES = 8
B = 131072
PER_CORE = B // NCORES          # 16384
P = 128
NT = PER_CORE // P              # 128 tiles per core
FEAT = 128
NGATE = 3 * 384                 # i, g, o only (f is unused: c0 == 0)
NUM_NODES = 100000
NUM_EDGES = 500000

# matmul input dtype: "bf16" (2x PE throughput, ~1e-3 rel err) or
# "f32r" (fp22 truncation, ~1e-4 rel err)
MM_DTYPE = os.environ.get("KERNEL_MM_DTYPE", "f32r")

LAST_EXEC_NS = None

_PROGRAM_CACHE = {}


def _build_program(can_fold: bool):
    """Build the SPMD Bass program (shared by all 8 cores)."""
    dt_f32 = mybir.dt.float32
    dt_i32 = mybir.dt.int32
    if MM_DTYPE == "bf16":
        dt_mm = mybir.dt.bfloat16
        dt_w = mybir.dt.bfloat16
    else:
        dt_mm = mybir.dt.float32r
        dt_w = mybir.dt.float32r

    nc = bacc.Bacc("TRN2", target_bir_lowering=False, debug=False,
                   num_devices=NCORES)

    node_d = nc.dram_tensor("node_feat", [NUM_NODES, FEAT], dt_f32,
                            kind="ExternalInput").ap()
    edge_d = nc.dram_tensor("edge_feat", [NUM_EDGES, FEAT], dt_f32,
                            kind="ExternalInput").ap()
    src_d = nc.dram_tensor("src_i", [P, NT], dt_i32, kind="ExternalInput").ap()
    tgt_d = nc.dram_tensor("tgt_i", [P, NT], dt_i32, kind="ExternalInput").ap()
    eid_d = nc.dram_tensor("e_i", [P, NT], dt_i32, kind="ExternalInput").ap()
    ct_d = nc.dram_tensor("ct_rows", [NT, P], dt_f32, kind="ExternalInput").ap()
    w_d = nc.dram_tensor("wT", [384, NGATE], dt_w, kind="ExternalInput").ap()
    fr_d = nc.dram_tensor("freq", [P, 1], dt_f32, kind="ExternalInput").ap()
    ph_d = nc.dram_tensor("ph2", [P, 1], dt_f32, kind="ExternalInput").ap()
    if not can_fold:
        bias_d = nc.dram_tensor("bias_row", [1, NGATE], dt_f32,
                                kind="ExternalInput").ap()
    h_d = nc.dram_tensor("h_out", [PER_CORE, 384], dt_f32,
                         kind="ExternalOutput").ap()
    c_d = nc.dram_tensor("c_out", [PER_CORE, 384], dt_f32,
                         kind="ExternalOutput").ap()

    def mm_ap(ap):
        return ap

    with tile.TileContext(nc) as tc:
        with (
            tc.tile_pool(name="const", bufs=1) as cpool,
            tc.tile_pool(name="work", bufs=3) as wpool,
            tc.tile_pool(name="gates_sb", bufs=2) as gpool,
            tc.tile_pool(name="psum_tr", bufs=2, space="PSUM") as ptr_pool,
            tc.tile_pool(name="psum_mm", bufs=2, space="PSUM") as pmm_pool,
        ):
            # ---- one-time loads ----
            idx_src = cpool.tile([P, NT], dt_i32)
            idx_tgt = cpool.tile([P, NT], dt_i32)
            idx_e = cpool.tile([P, NT], dt_i32)
            nc.sync.dma_start(out=idx_src[:], in_=src_d[:])
            nc.sync.dma_start(out=idx_tgt[:], in_=tgt_d[:])
            nc.sync.dma_start(out=idx_e[:], in_=eid_d[:])

            w_k = []
            for k in range(3):
                wk = cpool.tile([P, NGATE], dt_w, tag=f"w{k}")
                nc.sync.dma_start(out=wk[:], in_=w_d[k * P:(k + 1) * P, :])
                w_k.append(wk)

            freq_t = cpool.tile([P, 1], dt_f32)
            ph2_t = cpool.tile([P, 1], dt_f32)
            nc.sync.dma_start(out=freq_t[:], in_=fr_d[:])
            nc.sync.dma_start(out=ph2_t[:], in_=ph_d[:])

            ident = cpool.tile([P, P], dt_f32)
            make_identity(nc, ident[:])

            if not can_fold:
                bias_b = cpool.tile([P, NGATE], dt_f32)
                nc.sync.dma_start(out=bias_b[:],
                                  in_=bias_d[0:1, :].to_broadcast((P, NGATE)))

            # ---- per-tile pipeline ----
            h2 = None
            c2 = None
            for t in range(NT):
                # gathers (Pool / SWDGE)
                g_src = wpool.tile([P, FEAT], dt_f32, tag="g_src")
                g_tgt = wpool.tile([P, FEAT], dt_f32, tag="g_tgt")
                g_edge = wpool.tile([P, FEAT], dt_f32, tag="g_edge")
                nc.gpsimd.indirect_dma_start(
                    out=g_src[:], out_offset=None, in_=node_d[:],
                    in_offset=bass.IndirectOffsetOnAxis(
                        ap=idx_src[:, t:t + 1], axis=0))
                nc.gpsimd.indirect_dma_start(
                    out=g_tgt[:], out_offset=None, in_=node_d[:],
                    in_offset=bass.IndirectOffsetOnAxis(
                        ap=idx_tgt[:, t:t + 1], axis=0))
                nc.gpsimd.indirect_dma_start(
                    out=g_edge[:], out_offset=None, in_=edge_d[:],
                    in_offset=bass.IndirectOffsetOnAxis(
                        ap=idx_e[:, t:t + 1], axis=0))

                # ct broadcast [1,128] dram -> [128,128] sbuf
                ct_b = wpool.tile([P, P], dt_f32, tag="ct_b")
                nc.sync.dma_start(out=ct_b[:],
                                  in_=ct_d[t:t + 1, :].to_broadcast((P, P)))

                # hidden = node[src] + node[tgt]
                hid = wpool.tile([P, FEAT], dt_f32, tag="hid")
                nc.vector.tensor_tensor(out=hid[:], in0=g_src[:], in1=g_tgt[:],
                                        op=mybir.AluOpType.add)

                # transpose hid and edge into PSUM (PE), copy to SBUF aggT
                ps_tr = ptr_pool.tile([P, 2 * P], dt_f32, tag="ps_tr")
                nc.tensor.transpose(out=ps_tr[:, 0:P], in_=hid[:],
                                    identity=ident[:])
                nc.tensor.transpose(out=ps_tr[:, P:2 * P], in_=g_edge[:],
                                    identity=ident[:])

                aggT = wpool.tile([P, 3 * P], dt_mm, tag="aggT")
                nc.vector.tensor_copy(out=aggT[:, 0:2 * P], in_=ps_tr[:])
                # time block: sin(freq*ct + phase + pi/2), transposed layout
                nc.scalar.activation(
                    out=aggT[:, 2 * P:3 * P], in_=ct_b[:],
                    func=mybir.ActivationFunctionType.Sin,
                    bias=ph2_t[:], scale=freq_t[:])

                # matmuls: gates[b, n] += aggT_k.T @ w_k ; banks i|g|o at
                # 512-col offsets inside one 3-bank psum tile
                ps_g = pmm_pool.tile([P, 1536], dt_f32, tag="ps_g")
                for k in range(3):
                    lhsT = aggT[:, k * P:(k + 1) * P]
                    for n in range(3):
                        nc.tensor.matmul(
                            out=ps_g[:, n * 512:n * 512 + 384],
                            lhsT=mm_ap(lhsT),
                            rhs=mm_ap(w_k[k][:, n * 384:(n + 1) * 384]),
                            start=(k == 0), stop=(k == 2))

                if not can_fold:
                    gv = ps_g[:].rearrange("p (b x) -> p b x", x=512)[:, :, 0:384]
                    bv = bias_b[:].rearrange("p (b x) -> p b x", x=384)
                    nc.vector.tensor_tensor(out=gv, in0=gv, in1=bv,
                                            op=mybir.AluOpType.add)

                # activations
                sio = gpool.tile([P, 2, 384], dt_f32, tag="sio")
                ps_view = ps_g[:].rearrange("p (b x) -> p b x", x=512)
                nc.scalar.activation(
                    out=sio[:], in_=ps_view[:, 0::2, 0:384],
                    func=mybir.ActivationFunctionType.Sigmoid)
                tg = gpool.tile([P, 384], dt_f32, tag="tg")
                nc.scalar.activation(
                    out=tg[:], in_=ps_g[:, 512:896],
                    func=mybir.ActivationFunctionType.Tanh)

                if t % 2 == 0:
                    h2 = gpool.tile([P, 2, 384], dt_f32, tag="h2")
                    c2 = gpool.tile([P, 2, 384], dt_f32, tag="c2")
                half = t % 2

                c_t = c2[:, half, :]
                nc.vector.tensor_tensor(out=c_t, in0=sio[:, 0, :], in1=tg[:],
                                        op=mybir.AluOpType.mult)
                tc_t = gpool.tile([P, 384], dt_f32, tag="tc")
                nc.scalar.activation(out=tc_t[:], in_=c_t,
                                     func=mybir.ActivationFunctionType.Tanh)
                nc.vector.tensor_tensor(out=h2[:, half, :], in0=sio[:, 1, :],
                                        in1=tc_t[:], op=mybir.AluOpType.mult)

                if half == 1:
                    t0 = t - 1
                    h_slice = h_d[t0 * P:(t0 + 2) * P, :]
                    c_slice = c_d[t0 * P:(t0 + 2) * P, :]
                    nc.sync.dma_start(
                        out=h_slice.rearrange("(g p) d -> p g d", p=P),
                        in_=h2[:])
                    nc.sync.dma_start(
                        out=c_slice.rearrange("(g p) d -> p g d", p=P),
                        in_=c2[:])

    nc.compile()
    return nc


def _prepare_host(inputs):
    """Shard + preprocess inputs; returns (can_fold, in_maps, unshard info)."""
    src_idx = np.asarray(inputs["src_idx"]).astype(np.int32).ravel()
    tgt_idx = np.asarray(inputs["tgt_idx"]).astype(np.int32).ravel()
    e_idx = np.asarray(inputs["e_idx"]).astype(np.int32).ravel()
    cut_time = np.asarray(inputs["cut_time"], dtype=np.float32).ravel()
    node_feat = np.ascontiguousarray(np.asarray(inputs["node_feat"],
                                                dtype=np.float32))
    edge_feat = np.ascontiguousarray(np.asarray(inputs["edge_feat"],
                                                dtype=np.float32))
    basis_freq = np.asarray(inputs["basis_freq"], dtype=np.float32).ravel()
    phase = np.asarray(inputs["phase"], dtype=np.float32).ravel()
    w_ih = np.asarray(inputs["w_ih"], dtype=np.float32)
    b_ih = np.asarray(inputs["b_ih"], dtype=np.float32).ravel()
    b_hh = np.asarray(inputs["b_hh"], dtype=np.float32).ravel()

    M = 384
    # gate rows i, g, o of w_ih (f dropped: f*c0 == 0)
    w_sel = np.concatenate([w_ih[0:M], w_ih[2 * M:3 * M], w_ih[3 * M:4 * M]],
                           axis=0)                       # [1152, 384]
    bias = np.concatenate([(b_ih + b_hh)[0:M], (b_ih + b_hh)[2 * M:3 * M],
                           (b_ih + b_hh)[3 * M:4 * M]])  # [1152]
    wT = np.ascontiguousarray(w_sel.T)                   # [384, 1152]

    # bias fold: time-encode rows whose cos() == 1.0 to fp32 accuracy for
    # every input: |freq_k * ct + phase_k| small for all ct in batch.
    ct_max = float(np.abs(cut_time).max())
    margin = np.abs(basis_freq) * ct_max + np.abs(phase)
    foldable = margin < 1e-4          # cos err < 5e-9
    can_fold = bool(foldable.any())
    time_rows = wT[128:256].copy()
    if can_fold:
        k_fold = int(np.argmax(foldable))
        folded_rows = np.where(foldable)[0]
        acc = time_rows[folded_rows].sum(axis=0) + bias
        time_rows[folded_rows] = 0.0
        time_rows[k_fold] = acc
    # device aggT feature order is [node | edge | time]
    wT_f = np.concatenate([wT[0:128], wT[256:384], time_rows], axis=0)

    if MM_DTYPE == "bf16":
        import ml_dtypes
        wT_dev = wT_f.astype(ml_dtypes.bfloat16)
    else:
        wT_dev = wT_f

    ph2 = (phase + np.float32(np.pi / 2)).astype(np.float32)

    in_maps = []
    for k in range(NCORES):
        sl = slice(k * PER_CORE, (k + 1) * PER_CORE)
        in_maps.append({
            "node_feat": node_feat,
            "edge_feat": edge_feat,
            "src_i": np.ascontiguousarray(
                src_idx[sl].reshape(NT, P).T),
            "tgt_i": np.ascontiguousarray(
                tgt_idx[sl].reshape(NT, P).T),
            "e_i": np.ascontiguousarray(
                e_idx[sl].reshape(NT, P).T),
            "ct_rows": np.ascontiguousarray(
                cut_time[sl].reshape(NT, P)),
            "wT": wT_dev,
            "freq": basis_freq.reshape(P, 1).copy(),
            "ph2": ph2.reshape(P, 1).copy(),
            **({} if can_fold else {"bias_row": bias.reshape(1, -1).copy()}),
        })
    return can_fold, in_maps


def kernel(**inputs):
    global LAST_EXEC_NS
    can_fold, in_maps = _prepare_host(inputs)

    key = (can_fold, MM_DTYPE)
    if key not in _PROGRAM_CACHE:
        _PROGRAM_CACHE[key] = _build_program(can_fold)
    nc = _PROGRAM_CACHE[key]

    trace = os.environ.get("KERNEL_TRACE", "0") == "1"
    res = run_bass_kernel_spmd(nc, in_maps, list(range(NCORES)), trace=trace)
    LAST_EXEC_NS = res.exec_time_ns

    h = np.empty((B, 384), dtype=np.float32)
    c = np.empty((B, 384), dtype=np.float32)
    for k in range(NCORES):
        sl = slice(k * PER_CORE, (k + 1) * PER_CORE)
        h[sl] = res.results[k]["h_out"]
        c[sl] = res.results[k]["c_out"]
    return h, c
